# revision 16
# baseline (speedup 1.0000x reference)
"""MoE routing kernel for Trainium2 (8 NeuronCores).

Problem: out[b,l,:] = actions[b,l,:]                      if action_type[b,l] == 0
         out[b,l,:] = W[t-1] @ actions[b,l,:] + b[t-1]    if action_type == t >= 1

Strategy (bf16, balanced expert-split): route each token to the single
expert it needs. The host groups the B*L tokens by action_type and splits
the 7 experts' token sets across all 8 cores: every core runs the same
SPMD program with token segments A/B/C, each segment served by its own
expert weight. Slot layout is chosen dynamically from the actual type
counts: (8,4,2) blocks = 1792 tokens/core when the distribution allows it
with a small host spill, else (8,4,3) = 1920. Identity tokens are copied
on the host (exact); tokens beyond device capacity fall back to host BLAS.

Device schedule: token tiles in two 512-feature phases of up to 8 PSUM
groups [128 tok, 512 feat], accumulated over 8 contraction chunks. DMA
supply ramps slowly (~120 GB/s early -> ~430 GB/s) and in-flight
descriptors share bandwidth round-robin at packet granularity, so the
preamble streams the first-needed pieces in exact consumption order and
defers the later expert weights (segment 2 dispatches ride behind the
first output store's semaphore wait) to keep early bandwidth for the
first tile. A warmup chain of dummy matmuls (vector-engine memsets, no
DMA deps) covers the PE p-state ramp (2x slower until ~3us of continuous
execution). PSUM->SBUF bf16 casts alternate vector/scalar; outputs leave
as paired [128, 2048] DMAs.
"""

import sys

for _p in ("/root/.axon_site/_ro/trn_rl_repo", "/opt/trn_rl_repo"):
    if _p not in sys.path:
        sys.path.append(_p)

import numpy as np
import ml_dtypes
import concourse.bass as bass
import concourse.tile as tile
from concourse import bacc, mybir
from concourse.bass_utils import run_bass_kernel_spmd

D = 1024
P = 128
N_CORES = 8
FB = 512  # psum feature block (phase width)
NIC = D // P  # 8 contraction chunks
NPH = D // FB  # 2 feature phases
F32 = mybir.dt.float32
BF16 = mybir.dt.bfloat16
BF16NP = ml_dtypes.bfloat16

# Candidate balanced layouts: (slot blocks per core, tile spec).
SEGS14 = ((0, 1024, 0), (1024, 512, 1), (1536, 128, 2), (1664, 128, 2))
SEGS15 = ((0, 1024, 0), (1024, 512, 1), (1536, 256, 2), (1792, 128, 2))
SLOTS14 = (8, 4, 2)
SLOTS15 = (8, 4, 3)
MAX_SPILL = 384  # max tokens sent to host BLAS to enable the smaller layout

_program_cache: dict[tuple, bass.Bass] = {}


def _t_tiles(C):
    """Decreasing tile sizes for the fallback one-expert-per-core path."""
    tiles = []
    t0 = 0
    while t0 < C:
        rest = C - t0
        if rest > 1536:
            tt = 1024
        elif rest > 768:
            tt = 512
        elif rest > 384:
            tt = 256
        else:
            tt = min(P, rest)
        tiles.append((t0, tt, 0))
        t0 += tt
    return tiles


def build_program(tile_spec, n_wseg: int, with_bias: bool) -> bass.Bass:
    """out = x @ w[seg].T per-core, x/w host-packed bf16.

    tile_spec: tuple of (t0, tt, wseg) token tiles.
    DRAM inputs per core:
      xP [P, NIC*C]    : cols [(NIC*t0 + ic*tt) ...] hold
                         x.T[ic*128:(ic+1)*128, t0:t0+tt]  (contract chunk
                         ic, token tile [t0, t0+tt)) -- SBUF-ready.
      wP [P, ws*NIC*D] : cols [ws*NIC*D + ph*NIC*FB + ic*FB ...] =
                         w[ws].T[ic*128:(ic+1)*128, ph*FB:(ph+1)*FB]
      bB [P, ws*D]     : broadcast bias rows per segment (if with_bias)
    DRAM output: outP [P, C*D/P] bf16, partition-major: token g*128+p,
    feature f lives at outP[p, g*D + f]. Host unpacks.
    """
    tile_spec = tuple(tile_spec)
    C = sum(tt for _, tt, _ in tile_spec)
    key = (tile_spec, n_wseg, with_bias)
    if key in _program_cache:
        return _program_cache[key]
    nc = bacc.Bacc("TRN2", target_bir_lowering=False, debug=False, num_devices=N_CORES)
    xP = nc.dram_tensor("xP", [P, NIC * C], BF16, kind="ExternalInput")
    wP = nc.dram_tensor("wP", [P, n_wseg * NIC * D], BF16, kind="ExternalInput")
    bB = (
        nc.dram_tensor("bB", [P, n_wseg * D], F32, kind="ExternalInput")
        if with_bias
        else None
    )
    outP = nc.dram_tensor("outP", [P, (C // P) * D], BF16, kind="ExternalOutput")

    PHW = NIC * FB  # wP columns per phase (4096)
    last_ti = len(tile_spec) - 1

    with tile.TileContext(nc) as tc:
        with (
            tc.tile_pool(name="wpool", bufs=1) as wpool,
            tc.tile_pool(name="bpool", bufs=1) as bpool,
            tc.tile_pool(name="xpool", bufs=2) as xpool,
            tc.tile_pool(name="opool", bufs=2) as opool,
            tc.tile_pool(name="psum", bufs=1, space="PSUM") as psum_pool,
        ):
            tt0 = tile_spec[0][1]
            w_tiles = {}  # (wseg, phase, ic) -> (tile, col offset)
            x0_tiles = [None] * NIC

            # PE warmup: dummy matmuls (no DMA deps) during the DMA preamble
            # so the p-state ramp (2x slower until ~3us continuous) completes
            # before real operands land. Memsets go on the idle vector engine
            # so the chain starts right at the TileContext entry. Two
            # rotating PSUM banks, start/stop once per bank: consecutive
            # same-address starts serialize on the PSUM reset (~430ns).
            warm = wpool.tile([P, FB], BF16, name="warm")
            warm2 = wpool.tile([P, P], BF16, name="warm2")
            nc.vector.memset(warm2[:], 0.0)
            nc.vector.memset(warm[:], 0.0)
            ps_warms = [
                psum_pool.tile([P, FB], F32, name=f"ps_warm{j}", tag=f"ps{6 + j}")
                for j in range(2)
            ]
            # Warmup: 6 full-width matmuls cover the p-state ramp (2x slower
            # until ~3us of continuous execution; an idle gap resets the
            # streak). With the split first x piece the real operands land
            # before the chain ends, so the streak carries straight into the
            # real matmuls. Each bank's group opens/closes once (a
            # same-address restart serializes on the PSUM reset, ~430ns).
            NWARM = 6
            for i in range(NWARM):
                nc.tensor.matmul(
                    ps_warms[i % 2][:],
                    warm2[:],
                    warm[:],
                    start=(i < 2),
                    stop=(i >= NWARM - 2),
                )

            def _dma_w(ws, ph, ic0_, nic_, engine=None):
                wt = wpool.tile(
                    [P, nic_ * FB],
                    BF16,
                    name=f"w{ws}_{ph}_{ic0_}",
                    tag=f"w{ws}_{ph}_{ic0_}",
                )
                base = ws * NIC * D + ph * PHW + ic0_ * FB
                (engine or nc.sync).dma_start(wt[:], wP[:, base : base + nic_ * FB])
                for j in range(nic_):
                    w_tiles[(ws, ph, ic0_ + j)] = (wt, j * FB)

            x0_csplit = {}  # ic -> list of (tile, col_off, c_lo, c_hi)

            def _dma_x0(ic0_, nic_):
                xt = xpool.tile(
                    [P, nic_ * tt0], BF16, name=f"x0_{ic0_}", tag=f"x0_{ic0_}"
                )
                nc.scalar.dma_start(xt[:], xP[:, ic0_ * tt0 : (ic0_ + nic_) * tt0])
                for j in range(nic_):
                    x0_tiles[ic0_ + j] = (xt, j * tt0)

            def _dma_x0_half(ic0_, half):
                # half a contraction chunk: covers c in [half*ntc0/2, ...)
                hw = tt0 // 2
                xt = xpool.tile(
                    [P, hw], BF16, name=f"x0_{ic0_}h{half}", tag=f"x0_{ic0_}h{half}"
                )
                base = ic0_ * tt0 + half * hw
                nc.scalar.dma_start(xt[:], xP[:, base : base + hw])
                c0_ = half * (tt0 // P // 2)
                x0_csplit.setdefault(ic0_, []).append(
                    (xt, -c0_ * P, c0_, c0_ + tt0 // P // 2)
                )

            # Preamble ladder: first-needed pieces first, fine-grained so
            # completion order tracks consumption order despite the packet
            # round-robin across in-flight descriptors. Weights on the sync
            # HWDGE queue, first-tile x on the scalar HWDGE queue. Everything
            # not needed for tile 0 is deferred behind the first out-store.
            _dma_w(0, 0, 0, 1)
            _dma_x0_half(0, 0)
            _dma_x0_half(0, 1)
            _dma_w(0, 0, 1, 1)
            _dma_x0(1, 1)
            _dma_w(0, 0, 2, 2)
            _dma_x0(2, 2)
            _dma_w(0, 0, 4, 2)
            _dma_x0(4, 2)
            _dma_w(0, 0, 6, 2)
            _dma_x0(6, 2)
            _dma_w(0, 1, 0, 4)
            _dma_w(0, 1, 4, 4)
            b_tile = None
            if with_bias:
                b_tile = bpool.tile([P, n_wseg * D], F32, name="b_tile")
                nc.sync.dma_start(b_tile[:], bB[:])

            _x_prefetched = {}
            for ti, (t0, tt, ws) in enumerate(tile_spec):
                ntc = tt // P  # token chunks in this tile (<= 8)
                if ti == 0:
                    xv = x0_tiles
                else:
                    xv = _x_prefetched.pop(ti)

                ot = opool.tile([P, ntc * D], BF16, name=f"ot_{ti}", tag=f"o{ti % 2}")
                for ph in range(NPH):
                    if ti >= 1 and ph == 1 and ti + 1 < len(tile_spec):
                        # prefetch next tile's x on the sync queue BEFORE
                        # this tile's out dispatches enter that queue
                        # (tile 1's x is deferred to the first-out hook)
                        nt0, ntt, _ = tile_spec[ti + 1]
                        xt = xpool.tile(
                            [P, NIC * ntt],
                            BF16,
                            name=f"x{ti + 1}",
                            tag=f"x{(ti + 1) % 2}",
                        )
                        base = NIC * nt0
                        nc.sync.dma_start(xt[:], xP[:, base : base + NIC * ntt])
                        _x_prefetched[ti + 1] = [
                            (xt, j * ntt) for j in range(NIC)
                        ]
                    ps = {
                        c: psum_pool.tile(
                            [P, FB],
                            F32,
                            name=f"ps_{ti}_{ph}_{c}",
                            tag=f"ps{(ph * ntc + c) % 8}",
                        )
                        for c in range(ntc)
                    }
                    for ic in range(NIC):
                        last = ic == NIC - 1
                        wt, woff = w_tiles[(ws, ph, ic)]
                        rhs = wt[:, woff : woff + FB]
                        for c in range(ntc):
                            if ti == 0 and ic in x0_csplit:
                                for piece in x0_csplit[ic]:
                                    if piece[2] <= c < piece[3]:
                                        xt, xoff = piece[0], piece[1]
                                        break
                            else:
                                xt, xoff = xv[ic]
                            nc.tensor.matmul(
                                ps[c][:],
                                xt[:, xoff + c * P : xoff + (c + 1) * P],
                                rhs,
                                start=(ic == 0),
                                stop=last,
                            )
                            if last:
                                # evacuate psum as soon as its group closes,
                                # alternating engines
                                dst = ot[:, c * D + ph * FB : c * D + (ph + 1) * FB]
                                final_chunk = (
                                    ti == last_ti and c == ntc - 1 and not with_bias
                                )
                                if with_bias:
                                    nc.vector.tensor_add(
                                        dst,
                                        ps[c][:],
                                        b_tile[:, ws * D + ph * FB : ws * D + (ph + 1) * FB],
                                    )
                                elif final_chunk:
                                    # final drain: vector-only cast (the
                                    # scalar engine is usually parked on a
                                    # pool wait here and would start late),
                                    # then store this phase's half right away
                                    # so the exit barrier waits on a minimal
                                    # transfer
                                    nc.vector.tensor_copy(dst, ps[c][:])
                                    g = t0 // P + c
                                    nc.sync.dma_start(
                                        outP[:, g * D + ph * FB : g * D + (ph + 1) * FB],
                                        dst,
                                    )
                                elif c % 2 == 0:
                                    nc.vector.tensor_copy(dst, ps[c][:])
                                else:
                                    nc.scalar.copy(dst, ps[c][:])
                                if (
                                    ph == NPH - 1
                                    and c % 2 == 1
                                    and not (ti == last_ti and c == ntc - 1 and not with_bias)
                                ):
                                    # store chunk pair (c-1, c)
                                    g = t0 // P + c - 1
                                    nc.sync.dma_start(
                                        outP[:, g * D : (g + 2) * D],
                                        ot[:, (c - 1) * D : (c + 1) * D],
                                    )
                                    if ti == 0 and c == 1:
                                        # deferred bulk: this point in the
                                        # sync queue sits behind the first
                                        # out-store's semaphore wait (fires
                                        # ~tile-0 ph1), so tile-1's x and the
                                        # later expert weights don't dilute
                                        # the bandwidth-starved early window
                                        # where tile-0's pieces stream.
                                        if len(tile_spec) > 1:
                                            nt0, ntt, _ = tile_spec[1]
                                            xt = xpool.tile(
                                                [P, NIC * ntt],
                                                BF16,
                                                name="x1",
                                                tag="x1",
                                            )
                                            base = NIC * nt0
                                            nc.sync.dma_start(
                                                xt[:], xP[:, base : base + NIC * ntt]
                                            )
                                            _x_prefetched[1] = [
                                                (xt, j * ntt) for j in range(NIC)
                                            ]
                                        for ws2 in range(1, n_wseg):
                                            for ph2 in range(NPH):
                                                _dma_w(ws2, ph2, 0, 4)
                                                _dma_w(ws2, ph2, 4, 4)
                    if ph == NPH - 1 and ntc % 2 == 1:
                        g = t0 // P + ntc - 1
                        if not (ti == last_ti and not with_bias):
                            nc.sync.dma_start(
                                outP[:, g * D : (g + 1) * D],
                                ot[:, (ntc - 1) * D : ntc * D],
                            )
    nc.compile()
    _program_cache[key] = nc
    return nc


def _pack_x(flat_rows: np.ndarray, tile_spec) -> np.ndarray:
    """[n, D] fp32 tokens -> [P, NIC*C] bf16 in (tile, ic)-block layout."""
    C = sum(tt for _, tt, _ in tile_spec)
    n = flat_rows.shape[0]
    xT = np.zeros((D, C), dtype=np.float32)
    if n:
        xT[:, :n] = flat_rows.T
    xP = np.empty((P, NIC * C), dtype=BF16NP)
    for t0, tt, _ in tile_spec:
        base = NIC * t0
        for ic in range(NIC):
            xP[:, base + ic * tt : base + (ic + 1) * tt] = xT[
                ic * P : (ic + 1) * P, t0 : t0 + tt
            ].astype(BF16NP)
    return xP


def _pack_w(wTs) -> np.ndarray:
    """list of [D, D] fp32 w.T -> [P, len*NIC*D] bf16, phase-major."""
    PHW = NIC * FB
    wP = np.empty((P, len(wTs) * NIC * D), dtype=BF16NP)
    for ws, wT in enumerate(wTs):
        base = ws * NIC * D
        for ph in range(NPH):
            for ic in range(NIC):
                wP[:, base + ph * PHW + ic * FB : base + ph * PHW + (ic + 1) * FB] = (
                    wT[ic * P : (ic + 1) * P, ph * FB : (ph + 1) * FB].astype(BF16NP)
                )
    return wP


def _unpack_out(oP, tile_spec):
    """outP [P, C*D/P] bf16 (chunk-major) -> [C, D] float32."""
    C = sum(tt for _, tt, _ in tile_spec)
    return (
        oP.reshape(P, C // P, D).transpose(1, 0, 2).reshape(C, D).astype(np.float32)
    )


def _pack_slots(block_counts, slot_blocks):
    """DFS-pack per-expert block counts into 8 cores x slots slot_blocks.
    Returns per-expert list of (core, slot_idx, capacity_tokens), or None."""
    order = sorted(range(len(block_counts)), key=lambda i: -block_counts[i])
    s0, s1, s2 = slot_blocks

    def combos(n):
        out = []
        for a in range(9):
            for bq in range(9):
                for c in range(9):
                    cap = s0 * a + s1 * bq + s2 * c
                    if cap >= n and cap - n <= 6:
                        out.append((cap - n, a, bq, c))
        out.sort()
        return [(a, bq, c) for _, a, bq, c in out]

    assign = {}

    def dfs(k, rem):
        if k == len(order):
            return True
        n = block_counts[order[k]]
        if n == 0:
            assign[order[k]] = (0, 0, 0)
            return dfs(k + 1, rem)
        for a, bq, c in combos(n):
            if a <= rem[0] and bq <= rem[1] and c <= rem[2]:
                assign[order[k]] = (a, bq, c)
                if dfs(k + 1, (rem[0] - a, rem[1] - bq, rem[2] - c)):
                    return True
        return False

    if not dfs(0, (8, 8, 8)):
        return None
    free = {s: list(range(8)) for s in range(3)}
    pieces = [[] for _ in block_counts]
    for e in order:
        a, bq, c = assign[e]
        for s, cnt in ((0, a), (1, bq), (2, c)):
            for _ in range(cnt):
                core = free[s].pop(0)
                pieces[e].append((core, s, slot_blocks[s] * P))
    return pieces


def _choose_layout(counts):
    """Pick (tile_spec, slot_blocks, pieces) for the balanced path, trying
    the 14-block layout (with a small host spill) before the 15-block one.
    Returns None if neither packs (-> one-expert-per-core fallback)."""
    blocks = [-(-counts[t] // P) for t in range(1, N_CORES)]
    if sum(blocks) > 120:
        return None
    # try 14 blocks/core: truncate cheapest partial blocks until <= 112
    blocks14 = list(blocks)
    spill = 0
    while sum(blocks14) > 112:
        best, best_r = None, 1 << 30
        for t in range(len(blocks14)):
            if blocks14[t] == 0:
                continue
            r = counts[t + 1] - (blocks14[t] - 1) * P  # tokens in last block
            if r < best_r:
                best, best_r = t, r
        blocks14[best] -= 1
        spill += best_r
    if spill <= MAX_SPILL:
        pieces = _pack_slots(blocks14, SLOTS14)
        if pieces is not None:
            return SEGS14, SLOTS14, pieces
    pieces = _pack_slots(blocks, SLOTS15)
    if pieces is not None:
        return SEGS15, SLOTS15, pieces
    return None


def _run(nc, in_maps, trace):
    return run_bass_kernel_spmd(nc, in_maps, list(range(N_CORES)), trace=trace)


def kernel(actions, action_type, W, b, _trace=False):
    actions = np.ascontiguousarray(actions, dtype=np.float32)
    B, L, _ = actions.shape
    flat = actions.reshape(B * L, D)
    types = np.asarray(action_type).reshape(B * L).astype(np.int64)

    idx = [np.flatnonzero(types == t) for t in range(N_CORES)]
    counts = [len(i) for i in idx]

    W = np.asarray(W, dtype=np.float32)
    b_np = np.asarray(b, dtype=np.float32)
    with_bias = bool(np.any(b_np))
    wTs = [np.eye(D, dtype=np.float32)] + [W[t].T for t in range(N_CORES - 1)]

    layout = _choose_layout(counts)

    out_flat = np.empty_like(flat)
    out_flat[idx[0]] = flat[idx[0]]  # identity tokens: exact copy
    host_leftover = []  # (expert t, token indices) computed on host

    if layout is not None:
        tile_spec, slot_blocks, pieces = layout
        C_BAL = sum(tt for _, tt, _ in tile_spec)
        # slot s -> token offset of that segment within the core
        seg_off = {0: 0, 1: slot_blocks[0] * P, 2: (slot_blocks[0] + slot_blocks[1]) * P}
        core_rows = [np.zeros((C_BAL, D), np.float32) for _ in range(N_CORES)]
        core_wseg = [[0, 0, 0] for _ in range(N_CORES)]  # wT index per segment
        core_orig = [np.full(C_BAL, -1, np.int64) for _ in range(N_CORES)]
        for t in range(1, N_CORES):
            toks = idx[t]
            pos = 0
            for core, s, cap in pieces[t - 1]:
                take = min(cap, len(toks) - pos)
                if take <= 0:
                    continue
                o = seg_off[s]
                core_rows[core][o : o + take] = flat[toks[pos : pos + take]]
                core_orig[core][o : o + take] = toks[pos : pos + take]
                core_wseg[core][s] = t
                pos += take
            if pos < len(toks):
                host_leftover.append((t, toks[pos:]))
        nc = build_program(tile_spec, 3, with_bias)
        in_maps = []
        for core in range(N_CORES):
            m = {
                "xP": _pack_x(core_rows[core], tile_spec),
                "wP": _pack_w([wTs[core_wseg[core][s]] for s in range(3)]),
            }
            if with_bias:
                bb = np.zeros((P, 3 * D), np.float32)
                for s in range(3):
                    t = core_wseg[core][s]
                    if t >= 1:
                        bb[:, s * D : (s + 1) * D] = b_np[t - 1]
                m["bB"] = bb
            in_maps.append(m)
        r = _run(nc, in_maps, _trace)
        for core in range(N_CORES):
            o = _unpack_out(r.results[core]["outP"], tile_spec)
            valid = core_orig[core] >= 0
            out_flat[core_orig[core][valid]] = o[valid]
    else:
        # Fallback: one expert per core, core 0 runs dummy zeros.
        C = max(P, min(2048, -(-max(counts[1:]) // P) * P))
        tile_spec = tuple(_t_tiles(C))
        nc = build_program(tile_spec, 1, with_bias)
        in_maps = []
        for t in range(N_CORES):
            n_dev = 0 if t == 0 else min(counts[t], C)
            rows = np.zeros((C, D), np.float32)
            if n_dev:
                rows[:n_dev] = flat[idx[t][:n_dev]]
            m = {"xP": _pack_x(rows, tile_spec), "wP": _pack_w([wTs[t]])}
            if with_bias:
                bvec = np.zeros(D, dtype=np.float32) if t == 0 else b_np[t - 1]
                m["bB"] = np.ascontiguousarray(
                    np.broadcast_to(bvec, (P, D)), dtype=np.float32
                )
            in_maps.append(m)
            if t >= 1 and counts[t] > n_dev:
                host_leftover.append((t, idx[t][n_dev:]))
        r = _run(nc, in_maps, _trace)
        for t in range(1, N_CORES):
            n_dev = min(counts[t], C)
            if n_dev:
                o = _unpack_out(r.results[t]["outP"], tile_spec)
                out_flat[idx[t][:n_dev]] = o[:n_dev]

    for t, ov in host_leftover:
        out_flat[ov] = flat[ov] @ W[t - 1].T + b_np[t - 1]

    out = out_flat.reshape(B, L, D)
    if _trace:
        return out, r
    return out
